# revision 1
# baseline (speedup 1.0000x reference)
"""EpisodicMemory kernel for Trainium2, 8-core data-parallel.

Reference computation (per batch b, d=32, m=64 memory slots, 2 hops):
    M = vs[b]
    for hop:
        Rh[m,:] = R[b,hop,m] @ h[b,hop,m]                  # batched matvec
        z = [Rh*v, Rh*M, |Rh-v|, |Rh-M|]                   # [m, 4d]
        Z = tanh(z @ W1.T + b1) @ W2.T (+ b2: dropped — softmax-invariant)
        g = softmax(Z over m); o = sum_m ts[b,hop,m] * g[m]
        M = GRUCell(o, M)
    out[b] = M

Sharding: pure data parallel over batch; 128 batches per core.

Per-core layout strategy:
  - einsum Rh: R tiles [128 part=(m,bp), free=(g4,d32,e32)] (b = blk*8+bp*4+g;
    p = m*2+bp, so one 128-partition DMA per block covers 2 MB at full rate),
    DVE mul (in-place, h broadcast over d) + DVE reduce over e.
  - features built in row layout [128 rows, (g,f,d)], PE-transposed to
    z^T [feat128, rows] for the MLP matmuls on TensorE.
  - softmax/o batched per hop over all 128 batches [128 part=b, 64 m].
  - GRU in transposed layout [d part, b free]; M kept as MT [32,128] + M_row.
"""

import numpy as np

import concourse.bacc as bacc
import concourse.bass as bass
import concourse.mybir as mybir
import concourse.tile as tile
from concourse.masks import make_identity
from concourse.tile import add_dep_helper

F32 = mybir.dt.float32
AF = mybir.ActivationFunctionType
ALU = mybir.AluOpType
AX = mybir.AxisListType

B, N_HOP, N_MEM, DIM = 1024, 2, 64, 32
N_CORES = 8
BC = B // N_CORES            # 128 batches per core
BB = 8                       # batches per block
NBLK = BC // BB              # 16 blocks
NG = BB // 2                 # 4 b-pair groups per block
ROWS = BB * N_MEM            # 512 rows per block
D4 = 4 * DIM                 # 128 MLP input features


DEBUG = False


def build_nc(n_iter: int = 1) -> bass.Bass:
    nc = bacc.Bacc("TRN2")

    # Rs/hs arrive host-permuted: [hop, blk, m, bp, g, ...] with b = blk*8+bp*4+g
    Rs_d = nc.dram_tensor(
        "Rs", [N_HOP, NBLK, N_MEM, 2, NG, DIM, DIM], F32, kind="ExternalInput"
    )
    hs_d = nc.dram_tensor(
        "hs", [N_HOP, N_MEM, 2, NBLK, NG, DIM], F32, kind="ExternalInput"
    )
    ts_d = nc.dram_tensor("ts", [BC, N_HOP, N_MEM, DIM], F32, kind="ExternalInput")
    vs_d = nc.dram_tensor("vs", [BC, DIM], F32, kind="ExternalInput")
    W1_d = nc.dram_tensor("W1", [DIM, D4], F32, kind="ExternalInput")
    b1_d = nc.dram_tensor("b1", [DIM], F32, kind="ExternalInput")
    W2_d = nc.dram_tensor("W2", [1, DIM], F32, kind="ExternalInput")
    Wih_d = nc.dram_tensor("W_ih", [N_HOP, 3 * DIM, DIM], F32, kind="ExternalInput")
    Whh_d = nc.dram_tensor("W_hh", [N_HOP, 3 * DIM, DIM], F32, kind="ExternalInput")
    bih_d = nc.dram_tensor("b_ih", [N_HOP, 3 * DIM], F32, kind="ExternalInput")
    bhh_d = nc.dram_tensor("b_hh", [N_HOP, 3 * DIM], F32, kind="ExternalInput")
    out_d = nc.dram_tensor("out", [BC, DIM], F32, kind="ExternalOutput")
    m_scr = nc.dram_tensor("m_scratch", [BC, DIM], F32)  # internal DRAM bounce
    m_scr2 = nc.dram_tensor("m_scratch2", [2, NBLK, NG, DIM], F32)  # permuted
    m_scr3 = nc.dram_tensor("m_scratch3", [128, NBLK * NG * DIM], F32)
    v_scr = nc.dram_tensor("v_scratch", [2, NBLK, NG, DIM], F32)  # permuted vs
    v_scr3 = nc.dram_tensor("v_scratch3", [128, NBLK * NG * DIM], F32)
    # Z bounce, laid out so the gather is contiguous per natural batch index
    z_scr = nc.dram_tensor("z_scratch", [NBLK, 2, NG, N_MEM], F32)
    if DEBUG:
        dbg_rh = nc.dram_tensor("dbg_rh", [128, NG * DIM], F32, kind="ExternalOutput")
        dbg_vrep = nc.dram_tensor(
            "dbg_vrep", [128, NBLK * NG * DIM], F32, kind="ExternalOutput"
        )
        dbg_Z = nc.dram_tensor("dbg_Z", [BC, N_MEM], F32, kind="ExternalOutput")
        dbg_g = nc.dram_tensor("dbg_g", [BC, N_MEM], F32, kind="ExternalOutput")
        dbg_o = nc.dram_tensor("dbg_o", [BC, DIM], F32, kind="ExternalOutput")
        dbg_zt = nc.dram_tensor("dbg_zt", [D4, ROWS], F32, kind="ExternalOutput")

    import contextlib

    with tile.TileContext(nc) as tc:
        with (
            (tc.For_i(0, n_iter, 1) if n_iter > 1 else contextlib.nullcontext()),
            tc.tile_pool(name="consts", bufs=1) as consts,
            tc.tile_pool(name="hop_io", bufs=2) as hop_io,
            tc.tile_pool(name="rpool", bufs=4) as rpool,
            tc.tile_pool(name="fpool", bufs=3) as fpool,
            tc.tile_pool(name="zpool", bufs=3) as zpool,
            tc.tile_pool(name="apool", bufs=3) as apool,
            tc.tile_pool(name="small", bufs=2) as small,
            tc.tile_pool(name="mstate", bufs=2) as mstate,
            tc.tile_pool(name="pp_z", bufs=3, space="PSUM") as pp_z,
            tc.tile_pool(name="pp_1", bufs=2, space="PSUM") as pp_1,
            tc.tile_pool(name="pp_2", bufs=1, space="PSUM") as pp_2,
            tc.tile_pool(name="pp_g", bufs=2, space="PSUM") as pp_g,
        ):
            ident = consts.tile([128, 128], F32)
            make_identity(nc, ident)

            # ---- weights prep (one-time) ----
            w1_sb = consts.tile([DIM, D4], F32)
            nc.sync.dma_start(out=w1_sb, in_=W1_d[:, :])
            w1t_ps = pp_g.tile([D4, DIM], F32, tag="gpsum")
            nc.tensor.transpose(w1t_ps, w1_sb, ident[:DIM, :DIM])
            W1T = consts.tile([D4, DIM], F32)
            nc.scalar.copy(out=W1T, in_=w1t_ps)

            W2T = consts.tile([DIM, 1], F32)
            nc.sync.dma_start(out=W2T, in_=W2_d.rearrange("a b -> b a"))
            b1T = consts.tile([DIM, 1], F32)
            nc.sync.dma_start(out=b1T, in_=b1_d[:].unsqueeze(1))

            WihT, WhhT, bsum_rz, bihn_t, bhhn_t = [], [], [], [], []
            for hop in range(N_HOP):
                wih_sb = consts.tile([3 * DIM, DIM], F32, tag="wload", bufs=4)
                nc.sync.dma_start(out=wih_sb, in_=Wih_d[hop])
                wt_ps = pp_g.tile([DIM, 3 * DIM], F32, tag="gpsum")
                nc.tensor.transpose(wt_ps, wih_sb, ident[: 3 * DIM, : 3 * DIM])
                wT = consts.tile([DIM, 3 * DIM], F32, tag=f"wihT{hop}")
                nc.scalar.copy(out=wT, in_=wt_ps)
                WihT.append(wT)

                whh_sb = consts.tile([3 * DIM, DIM], F32, tag="wload", bufs=4)
                nc.sync.dma_start(out=whh_sb, in_=Whh_d[hop])
                wt_ps2 = pp_g.tile([DIM, 3 * DIM], F32, tag="gpsum")
                nc.tensor.transpose(wt_ps2, whh_sb, ident[: 3 * DIM, : 3 * DIM])
                wT2 = consts.tile([DIM, 3 * DIM], F32, tag=f"whhT{hop}")
                nc.scalar.copy(out=wT2, in_=wt_ps2)
                WhhT.append(wT2)

                # per-gate bias tiles, all at base partition 0
                gate_b = []
                for gd, gname in ((bih_d, "ih"), (bhh_d, "hh")):
                    for gate in range(3):
                        bt = consts.tile([DIM, 1], F32, tag=f"b{gname}{hop}{gate}")
                        nc.sync.dma_start(
                            out=bt,
                            in_=gd[hop, gate * DIM : (gate + 1) * DIM].unsqueeze(1),
                        )
                        gate_b.append(bt)
                b_r = consts.tile([DIM, 1], F32, tag=f"b_r{hop}")
                nc.vector.tensor_add(b_r, gate_b[0], gate_b[3])
                b_z = consts.tile([DIM, 1], F32, tag=f"b_z{hop}")
                nc.vector.tensor_add(b_z, gate_b[1], gate_b[4])
                bsum_rz.append((b_r, b_z))
                bihn_t.append(gate_b[2])
                bhhn_t.append(gate_b[5])

            # ---- initial M state ----
            vs_row = consts.tile([BC, DIM], F32)
            nc.sync.dma_start(out=vs_row, in_=vs_d[:, :])
            vst_ps = pp_g.tile([DIM, BC], F32, tag="gpsum")
            nc.tensor.transpose(vst_ps, vs_row, ident)
            vsT = consts.tile([DIM, BC], F32)
            nc.scalar.copy(out=vsT, in_=vst_ps)
            MT = vsT  # current M^T [d, b]

            # v_rep [128 part=(m,bp), (blk,g,d)]: value vs[blk*8+bp*4+g, d].
            # Stage a (bp, blk, g, d)-permuted copy in DRAM, then one plain
            # full-tile broadcast DMA (0-step m dim on the DRAM side).
            nc.sync.dma_start(
                out=v_scr[:, :, :, :],
                in_=vs_d.rearrange("(blk bp g) d -> bp blk g d", bp=2, g=NG),
            )
            # replicate to per-partition rows in DRAM (free-form APs), then a
            # plain [128, f] load (SBUF DMA APs must be partition-clean on HW)
            nc.sync.dma_start(
                out=v_scr3.rearrange("(m bp) f -> m bp f", bp=2),
                in_=v_scr.rearrange(
                    "bp blk g d -> bp (blk g d)"
                ).partition_broadcast(64),
            )
            v_rep = consts.tile([128, NBLK * NG * DIM], F32)
            prev_bcast_dma = nc.sync.dma_start(out=v_rep, in_=v_scr3[:, :])

            M_rep = v_rep  # hop 0: M == vs

            for hop in range(N_HOP):
                # per-hop h in einsum layout [(bp,m), (blk,g,e)]
                # h for the whole hop: one contiguous [128, 2048] load
                h_hop = hop_io.tile([128, NBLK * NG * DIM], F32, tag="h_hop")
                nc.scalar.dma_start(
                    out=h_hop,
                    in_=hs_d[hop].rearrange("m bp blk g e -> (m bp) (blk g e)"),
                )
                # t natural layout [b, (m,d)]
                t_hop = hop_io.tile([BC, N_MEM * DIM], F32, tag="t_hop")
                t_dma = nc.scalar.dma_start(
                    out=t_hop, in_=ts_d[:, hop].rearrange("b m d -> b (m d)")
                )
                # lane-ordering: keep the broadcast DMA strictly before t_hop
                add_dep_helper(t_dma.ins, prev_bcast_dma.ins,
                               reason="hwdge lane ordering")

                Z_row = small.tile([BC, N_MEM], F32, tag="Z_row")

                for blk in range(NBLK):
                    r_tile = rpool.tile([128, NG * DIM * DIM], F32, tag="R")
                    nc.sync.dma_start(
                        out=r_tile,
                        in_=Rs_d[hop, blk].rearrange(
                            "m bp g d e -> (m bp) (g d e)"
                        ),
                    )
                    # P = R * h (in-place), h broadcast over d
                    r4 = r_tile.rearrange("p (g d e) -> p g d e", g=NG, d=DIM)
                    h_v = (
                        h_hop[:, blk * NG * DIM : (blk + 1) * NG * DIM]
                        .rearrange("p (g e) -> p g e", g=NG)
                        .unsqueeze(2)
                        .broadcast_to((128, NG, DIM, DIM))
                    )
                    # odd blocks: mul on GPSIMD so it overlaps DVE reduces
                    # (only DVE's 2nd SBUF port is shared; reduce is 1-port)
                    if blk % 2 == 1:
                        nc.gpsimd.tensor_tensor(r4, r4, h_v, op=ALU.mult)
                    else:
                        nc.vector.tensor_mul(r4, r4, h_v)
                    # Rh[(bp,m), (g,d)] = sum_e P
                    rh = fpool.tile([128, NG * DIM], F32, tag="rh")
                    nc.vector.tensor_reduce(
                        out=rh,
                        in_=r_tile.rearrange("p (gd e) -> p gd e", e=DIM),
                        axis=AX.X,
                        op=ALU.add,
                    )
                    # features F [(bp,m), (g, f, d)]
                    f_blk = fpool.tile([128, NG * 4 * DIM], F32, tag="F")
                    f4 = f_blk.rearrange("p (g f d) -> p g f d", g=NG, f=4)
                    rh3 = rh.rearrange("p (g d) -> p g d", g=NG)
                    vr3 = v_rep[:, blk * NG * DIM : (blk + 1) * NG * DIM].rearrange(
                        "p (g d) -> p g d", g=NG
                    )
                    mr3 = M_rep[:, blk * NG * DIM : (blk + 1) * NG * DIM].rearrange(
                        "p (g d) -> p g d", g=NG
                    )
                    nc.vector.tensor_mul(f4[:, :, 0, :], rh3, vr3)
                    nc.vector.tensor_mul(f4[:, :, 1, :], rh3, mr3)
                    nc.vector.tensor_sub(f4[:, :, 2, :], rh3, vr3)
                    nc.vector.tensor_sub(f4[:, :, 3, :], rh3, mr3)
                    nc.scalar.activation(f4[:, :, 2, :], f4[:, :, 2, :], AF.Abs)
                    nc.scalar.activation(f4[:, :, 3, :], f4[:, :, 3, :], AF.Abs)

                    # transpose to z^T [(f,d), (g,bp,m)]
                    zt_ps = pp_z.tile([D4, ROWS], F32, tag="zt")
                    for g in range(NG):
                        nc.tensor.transpose(
                            zt_ps[:, g * 128 : (g + 1) * 128],
                            f_blk[:, g * 128 : (g + 1) * 128],
                            ident,
                        )
                    zt_sb = zpool.tile([D4, ROWS], F32, tag="zt_sb")
                    nc.scalar.copy(out=zt_sb, in_=zt_ps)

                    ps1 = pp_1.tile([DIM, ROWS], F32, tag="ps1")
                    nc.tensor.matmul(ps1, lhsT=W1T, rhs=zt_sb, start=True, stop=True)
                    a1 = apool.tile([DIM, ROWS], F32, tag="a1")
                    nc.scalar.activation(a1, ps1, AF.Tanh, bias=b1T)
                    ps2 = pp_2.tile([1, ROWS], F32, tag="ps2")
                    nc.tensor.matmul(ps2, lhsT=W2T, rhs=a1, start=True, stop=True)
                    z_sb = zpool.tile([1, ROWS], F32, tag="z_sb")
                    nc.scalar.copy(out=z_sb, in_=ps2)
                    if DEBUG and hop == 0 and blk == 0:
                        nc.sync.dma_start(out=dbg_rh[:, :], in_=rh)
                        nc.sync.dma_start(out=dbg_vrep[:, :], in_=v_rep)
                        nc.sync.dma_start(out=dbg_zt[:, :], in_=zt_sb)
                    # z_sb free order is (g, m, bp); store as (bp, g, m).
                    # src stays 1-partition (dim0 count 1): split by bp.
                    for bp in range(2):
                        nc.scalar.dma_start(
                            out=z_scr[blk, bp].unsqueeze(0),
                            in_=z_sb.rearrange("o (g m bp) -> o g m bp", g=NG, bp=2)[
                                :, :, :, bp
                            ],
                        )

                # gather Z rows from DRAM: flat (blk,bp,g) == natural b
                nc.scalar.dma_start(
                    out=Z_row,
                    in_=z_scr.rearrange("a b c m -> (a b c) m"),
                )

                # softmax over m, batched over all 128 b
                nmx = small.tile([BC, 1], F32, tag="nmx")
                nc.vector.tensor_reduce(
                    out=nmx, in_=Z_row, axis=AX.X, op=ALU.max, negate=True
                )
                e_row = small.tile([BC, N_MEM], F32, tag="e_row")
                nc.scalar.activation(e_row, Z_row, AF.Exp, bias=nmx)
                ssum = small.tile([BC, 1], F32, tag="ssum")
                nc.vector.tensor_reduce(out=ssum, in_=e_row, axis=AX.X, op=ALU.add)
                rsum = small.tile([BC, 1], F32, tag="rsum")
                nc.vector.reciprocal(rsum, ssum)
                g_row = small.tile([BC, N_MEM], F32, tag="g_row")
                nc.vector.tensor_scalar_mul(g_row, e_row, rsum)

                if DEBUG and hop == 0:
                    nc.sync.dma_start(out=dbg_Z[:, :], in_=Z_row)
                    nc.sync.dma_start(out=dbg_g[:, :], in_=g_row)

                # o[b,d] = sum_m t[b,m,d] * g[b,m]  (in-place mul into t_hop)
                t3 = t_hop.rearrange("b (m d) -> b m d", d=DIM)
                g3 = g_row.unsqueeze(2).broadcast_to((BC, N_MEM, DIM))
                nc.vector.tensor_mul(t3, t3, g3)
                o_row = small.tile([BC, DIM], F32, tag="o_row")
                nc.vector.tensor_reduce(
                    out=o_row,
                    in_=t_hop.rearrange("b (m d) -> b d m", d=DIM),
                    axis=AX.X,
                    op=ALU.add,
                )

                # GRU (transposed layout [*, b])
                if DEBUG and hop == 0:
                    nc.sync.dma_start(out=dbg_o[:, :], in_=o_row)

                ot_ps = pp_g.tile([DIM, BC], F32, tag="gpsum")
                nc.tensor.transpose(ot_ps, o_row, ident)
                oT = small.tile([DIM, BC], F32, tag="oT")
                nc.scalar.copy(out=oT, in_=ot_ps)

                # per-gate matmuls so every gate tile sits at base partition 0
                def gate_pair(g):
                    gi = pp_g.tile([DIM, BC], F32, tag="gpsum")
                    nc.tensor.matmul(
                        gi,
                        lhsT=WihT[hop][:, g * DIM : (g + 1) * DIM],
                        rhs=oT,
                        start=True,
                        stop=True,
                    )
                    gh = pp_g.tile([DIM, BC], F32, tag="gpsum")
                    nc.tensor.matmul(
                        gh,
                        lhsT=WhhT[hop][:, g * DIM : (g + 1) * DIM],
                        rhs=MT,
                        start=True,
                        stop=True,
                    )
                    return gi, gh

                # r,z gates: sigmoid(gi + gh + b_ih + b_hh)
                rz_t = []
                for g in range(2):
                    gi, gh = gate_pair(g)
                    gb = small.tile([DIM, BC], F32, tag=f"g{g}b")
                    nc.scalar.activation(gb, gi, AF.Identity, bias=bsum_rz[hop][g])
                    nc.vector.tensor_add(gb, gb, gh)
                    gt = small.tile([DIM, BC], F32, tag=f"gate{g}")
                    nc.scalar.activation(gt, gb, AF.Sigmoid)
                    rz_t.append(gt)
                r_t, z_t = rz_t

                # n = tanh(gi_n + b_ih_n + r * (gh_n + b_hh_n))
                gi_n, gh_n = gate_pair(2)
                ghn = small.tile([DIM, BC], F32, tag="ghn")
                nc.scalar.activation(ghn, gh_n, AF.Identity, bias=bhhn_t[hop])
                gin = small.tile([DIM, BC], F32, tag="gin")
                nc.scalar.activation(gin, gi_n, AF.Identity, bias=bihn_t[hop])
                n1 = small.tile([DIM, BC], F32, tag="n1")
                nc.vector.tensor_mul(n1, r_t, ghn)
                nc.vector.tensor_add(n1, n1, gin)
                n_t = small.tile([DIM, BC], F32, tag="n_t")
                nc.scalar.activation(n_t, n1, AF.Tanh)

                # M' = n + z * (M - n)
                MT_new = mstate.tile([DIM, BC], F32, tag="MT")
                nc.vector.tensor_sub(MT_new, MT, n_t)
                nc.vector.tensor_mul(MT_new, MT_new, z_t)
                nc.vector.tensor_add(MT_new, MT_new, n_t)
                MT = MT_new

                # M_row for output / M_rep rebuild
                mrow_ps = pp_g.tile([BC, DIM], F32, tag="gpsum")
                nc.tensor.transpose(mrow_ps, MT, ident[:DIM, :DIM])
                M_row = mstate.tile([BC, DIM], F32, tag="M_row")
                nc.scalar.copy(out=M_row, in_=mrow_ps)

                if hop < N_HOP - 1:
                    # rebuild M_rep via DRAM bounce
                    nc.scalar.dma_start(out=m_scr[:, :], in_=M_row)
                    nc.sync.dma_start(
                        out=m_scr2[:, :, :, :],
                        in_=m_scr.rearrange(
                            "(blk bp g) d -> bp blk g d", bp=2, g=NG
                        ),
                    )
                    M_rep_new = mstate.tile(
                        [128, NBLK * NG * DIM], F32, tag="M_rep", bufs=1
                    )
                    nc.sync.dma_start(
                        out=m_scr3.rearrange("(m bp) f -> m bp f", bp=2),
                        in_=m_scr2.rearrange(
                            "bp blk g d -> bp (blk g d)"
                        ).partition_broadcast(64),
                    )
                    prev_bcast_dma = nc.sync.dma_start(
                        out=M_rep_new, in_=m_scr3[:, :]
                    )
                    M_rep = M_rep_new
                else:
                    nc.sync.dma_start(out=out_d[:, :], in_=M_row)

    nc.compile()
    return nc


_NC_CACHE = None


def _get_nc():
    global _NC_CACHE
    if _NC_CACHE is None:
        _NC_CACHE = build_nc()
    return _NC_CACHE


def permute_local(x):
    """[BC, N_HOP, m, ...] -> [N_HOP, NBLK, m, 2, NG, ...] with b = blk*8+bp*4+g."""
    tail = x.shape[2:]
    y = x.reshape(NBLK, 2, NG, N_HOP, *tail)
    order = (3, 0, 4, 1, 2) + tuple(range(5, y.ndim))
    return np.ascontiguousarray(y.transpose(order))


def permute_h(x):
    """hs [BC, N_HOP, m, e] -> [N_HOP, m, 2, NBLK, NG, e]."""
    y = x.reshape(NBLK, 2, NG, N_HOP, N_MEM, DIM)
    return np.ascontiguousarray(y.transpose(3, 4, 1, 0, 2, 5))


def make_in_maps(hs, Rs, ts, vs, W1, b1, W2, W_ih, W_hh, b_ih, b_hh):
    in_maps = []
    for c in range(N_CORES):
        sl = slice(c * BC, (c + 1) * BC)
        in_maps.append(
            {
                "Rs": permute_local(Rs[sl]),
                "hs": permute_h(hs[sl]),
                "ts": np.ascontiguousarray(ts[sl]),
                "vs": np.ascontiguousarray(vs[sl]),
                "W1": np.ascontiguousarray(W1),
                "b1": np.ascontiguousarray(b1),
                "W2": np.ascontiguousarray(W2),
                "W_ih": np.ascontiguousarray(W_ih),
                "W_hh": np.ascontiguousarray(W_hh),
                "b_ih": np.ascontiguousarray(b_ih),
                "b_hh": np.ascontiguousarray(b_hh),
            }
        )
    return in_maps


def kernel(hs, Rs, ts, vs, W1, b1, W2, b2, W_ih, W_hh, b_ih, b_hh):
    from concourse.bass_utils import run_bass_kernel_spmd

    nc = _get_nc()
    in_maps = make_in_maps(hs, Rs, ts, vs, W1, b1, W2, W_ih, W_hh, b_ih, b_hh)
    res = run_bass_kernel_spmd(nc, in_maps, list(range(N_CORES)))
    return np.concatenate([r["out"] for r in res.results], axis=0)



# revision 5
# speedup vs baseline: 2.7050x; 2.7050x over previous
"""EpisodicMemory kernel for Trainium2, 8-core data-parallel, bf16 pipeline.

Reference computation (per batch b, d=32, m=64 memory slots, 2 hops):
    M = vs[b]
    for hop:
        Rh[m,:] = R[b,hop,m] @ h[b,hop,m]                  # batched matvec
        z = [Rh*v, Rh*M, |Rh-v|, |Rh-M|]                   # [m, 4d]
        Z = tanh(z @ W1.T + b1) @ W2.T (+ b2: dropped — softmax-invariant)
        g = softmax(Z over m); o = sum_m ts[b,hop,m] * g[m]
        M = GRUCell(o, M)
    out[b] = M

Sharding: pure data parallel over batch; 128 batches per core.

v2 strategy (vs f32 baseline):
  - Rs/hs/ts host-cast to bf16: halves HBM traffic; DVE tensor_tensor runs
    in 2x_1p mode (2 elem/cycle) on 16-bit operands with unit stride.
  - einsum: in-place bf16 mul (h broadcast over d, innermost e stride 1),
    then the e-sum as a 5-level pairwise add tree on DVE (2x mode) instead
    of TensorReduce (which has no fast modes); a subset of blocks' reduces
    run on GPSIMD tensor_reduce to balance engine load.
  - MLP matmuls + feature transposes in bf16 (PE fp32 matmul = 4 cyc/row,
    bf16 = 1 cyc/row).
  - Host pre-computes: layout permutes, W1T/W2T/WihT/WhhT transposes,
    vs_rep (vs replicated across m partitions) and vsT, so the device
    preamble has no PE transposes and no v_rep DRAM bounce.
  - ts shipped as [hop, b, d, m] so the o-sum mul (g broadcast over d) has
    innermost unit stride on both operands -> 2x mode.
  - GRU/softmax kept in f32 (tiny); M_rep rebuild bounce in bf16.
"""

import numpy as np
import ml_dtypes

import concourse.bacc as bacc
import concourse.bass as bass
import concourse.mybir as mybir
import concourse.tile as tile
from concourse.masks import make_identity
from concourse.tile import add_dep_helper

F32 = mybir.dt.float32
BF16 = mybir.dt.bfloat16
AF = mybir.ActivationFunctionType
ALU = mybir.AluOpType
AX = mybir.AxisListType

B, N_HOP, N_MEM, DIM = 1024, 2, 64, 32
N_CORES = 8
BC = B // N_CORES            # 128 batches per core
BB = 8                       # batches per block
NBLK = BC // BB              # 16 blocks
NG = BB // 2                 # 4 b-pair groups per block
ROWS = BB * N_MEM            # 512 rows per block
D4 = 4 * DIM                 # 128 MLP input features

# blocks whose e-reduce runs on GPSIMD (rest: DVE add-tree)
POOL_REDUCE = {2, 5, 8, 11, 14}


def build_nc(n_iter: int = 1) -> bass.Bass:
    nc = bacc.Bacc("TRN2")

    # Rs/hs arrive host-permuted: [hop, blk, m, bp, g, ...] with b = blk*8+bp*4+g
    Rs_d = nc.dram_tensor(
        "Rs", [N_HOP, NBLK, N_MEM, 2, NG, DIM, DIM], BF16, kind="ExternalInput"
    )
    hs_d = nc.dram_tensor(
        "hs", [N_HOP, N_MEM, 2, NBLK, NG, DIM], BF16, kind="ExternalInput"
    )
    # ts host-permuted to [hop, b, d, m]
    ts_d = nc.dram_tensor("ts", [N_HOP, BC, DIM, N_MEM], BF16, kind="ExternalInput")
    vsT_d = nc.dram_tensor("vsT", [DIM, BC], F32, kind="ExternalInput")
    vs_rep_d = nc.dram_tensor(
        "vs_rep", [128, NBLK * NG * DIM], BF16, kind="ExternalInput"
    )
    W1T_d = nc.dram_tensor("W1T", [D4, DIM], BF16, kind="ExternalInput")
    b1_d = nc.dram_tensor("b1", [DIM], F32, kind="ExternalInput")
    W2T_d = nc.dram_tensor("W2T", [DIM, 1], BF16, kind="ExternalInput")
    WihT_d = nc.dram_tensor("WihT", [N_HOP, DIM, 3 * DIM], F32, kind="ExternalInput")
    WhhT_d = nc.dram_tensor("WhhT", [N_HOP, DIM, 3 * DIM], F32, kind="ExternalInput")
    bih_d = nc.dram_tensor("b_ih", [N_HOP, 3 * DIM], F32, kind="ExternalInput")
    bhh_d = nc.dram_tensor("b_hh", [N_HOP, 3 * DIM], F32, kind="ExternalInput")
    out_d = nc.dram_tensor("out", [BC, DIM], F32, kind="ExternalOutput")
    m_scr = nc.dram_tensor("m_scratch", [BC, DIM], BF16)  # internal DRAM bounce
    m_scr2 = nc.dram_tensor("m_scratch2", [2, NBLK, NG, DIM], BF16)  # permuted
    m_scr3 = nc.dram_tensor("m_scratch3", [128, NBLK * NG * DIM], BF16)
    # Z bounce, laid out so the gather is contiguous per natural batch index
    z_scr = nc.dram_tensor("z_scratch", [NBLK, 2, NG, N_MEM], F32)

    import contextlib

    with tile.TileContext(nc) as tc:
        with (
            (tc.For_i(0, n_iter, 1) if n_iter > 1 else contextlib.nullcontext()),
            tc.tile_pool(name="consts", bufs=1) as consts,
            tc.tile_pool(name="hop_io", bufs=2) as hop_io,
            tc.tile_pool(name="rpool", bufs=4) as rpool,
            tc.tile_pool(name="tpool", bufs=3) as tpool,
            tc.tile_pool(name="fpool", bufs=3) as fpool,
            tc.tile_pool(name="zpool", bufs=3) as zpool,
            tc.tile_pool(name="apool", bufs=3) as apool,
            tc.tile_pool(name="small", bufs=2) as small,
            tc.tile_pool(name="mstate", bufs=2) as mstate,
            tc.tile_pool(name="pp_z", bufs=3, space="PSUM") as pp_z,
            tc.tile_pool(name="pp_1", bufs=2, space="PSUM") as pp_1,
            tc.tile_pool(name="pp_2", bufs=1, space="PSUM") as pp_2,
            tc.tile_pool(name="pp_g", bufs=2, space="PSUM") as pp_g,
        ):
            ident = consts.tile([128, 128], F32)
            make_identity(nc, ident)
            ident_bf = consts.tile([128, 128], BF16)
            nc.scalar.copy(out=ident_bf, in_=ident)

            # ---- weights (all pre-transposed on host) ----
            W1T = consts.tile([D4, DIM], BF16)
            nc.sync.dma_start(out=W1T, in_=W1T_d[:, :])
            W2T = consts.tile([DIM, 1], BF16)
            nc.sync.dma_start(out=W2T, in_=W2T_d[:, :])
            b1T = consts.tile([DIM, 1], F32)
            nc.sync.dma_start(out=b1T, in_=b1_d[:].unsqueeze(1))

            WihT, WhhT, bsum_rz, bihn_t, bhhn_t = [], [], [], [], []
            for hop in range(N_HOP):
                wT = consts.tile([DIM, 3 * DIM], F32, tag=f"wihT{hop}")
                nc.sync.dma_start(out=wT, in_=WihT_d[hop])
                WihT.append(wT)
                wT2 = consts.tile([DIM, 3 * DIM], F32, tag=f"whhT{hop}")
                nc.sync.dma_start(out=wT2, in_=WhhT_d[hop])
                WhhT.append(wT2)

                # per-gate bias tiles, all at base partition 0
                gate_b = []
                for gd, gname in ((bih_d, "ih"), (bhh_d, "hh")):
                    for gate in range(3):
                        bt = consts.tile([DIM, 1], F32, tag=f"b{gname}{hop}{gate}")
                        nc.sync.dma_start(
                            out=bt,
                            in_=gd[hop, gate * DIM : (gate + 1) * DIM].unsqueeze(1),
                        )
                        gate_b.append(bt)
                b_r = consts.tile([DIM, 1], F32, tag=f"b_r{hop}")
                nc.vector.tensor_add(b_r, gate_b[0], gate_b[3])
                b_z = consts.tile([DIM, 1], F32, tag=f"b_z{hop}")
                nc.vector.tensor_add(b_z, gate_b[1], gate_b[4])
                bsum_rz.append((b_r, b_z))
                bihn_t.append(gate_b[2])
                bhhn_t.append(gate_b[5])

            # ---- initial M state ----
            vsT = consts.tile([DIM, BC], F32)
            nc.sync.dma_start(out=vsT, in_=vsT_d[:, :])
            MT = vsT  # current M^T [d, b]

            v_rep = consts.tile([128, NBLK * NG * DIM], BF16)
            prev_bcast_dma = nc.sync.dma_start(out=v_rep, in_=vs_rep_d[:, :])

            M_rep = v_rep  # hop 0: M == vs

            for hop in range(N_HOP):
                # per-hop h in einsum layout [(m,bp), (blk,g,e)]
                h_hop = hop_io.tile([128, NBLK * NG * DIM], BF16, tag="h_hop")
                nc.scalar.dma_start(
                    out=h_hop,
                    in_=hs_d[hop].rearrange("m bp blk g e -> (m bp) (blk g e)"),
                )
                # t layout [b, (d, m)]
                t_hop = hop_io.tile([BC, DIM * N_MEM], BF16, tag="t_hop")
                t_dma = nc.scalar.dma_start(
                    out=t_hop, in_=ts_d[hop].rearrange("b d m -> b (d m)")
                )
                # lane-ordering: keep the broadcast DMA strictly before t_hop
                add_dep_helper(t_dma.ins, prev_bcast_dma.ins,
                               reason="hwdge lane ordering")

                Z_row = small.tile([BC, N_MEM], F32, tag="Z_row")

                for blk in range(NBLK):
                    r_tile = rpool.tile([128, NG * DIM * DIM], BF16, tag="R")
                    nc.sync.dma_start(
                        out=r_tile,
                        in_=Rs_d[hop, blk].rearrange(
                            "m bp g d e -> (m bp) (g d e)"
                        ),
                    )
                    # P = R * h (in-place), h broadcast over d (middle dim)
                    r4 = r_tile.rearrange("p (g d e) -> p g d e", g=NG, d=DIM)
                    h_v = (
                        h_hop[:, blk * NG * DIM : (blk + 1) * NG * DIM]
                        .rearrange("p (g e) -> p g e", g=NG)
                        .unsqueeze(2)
                        .broadcast_to((128, NG, DIM, DIM))
                    )
                    # odd blocks: mul on GPSIMD so it overlaps DVE tree-adds
                    if blk % 2 == 1:
                        nc.gpsimd.tensor_tensor(r4, r4, h_v, op=ALU.mult)
                    else:
                        nc.vector.tensor_mul(r4, r4, h_v)

                    # Rh[(m,bp), (g,d)] = sum_e P:
                    # 5-level pairwise add tree over e (bf16 2x mode on DVE)
                    rh = fpool.tile([128, NG * DIM], BF16, tag="rh")
                    tscr = tpool.tile([128, 4096], BF16, tag="tree")
                    lv_in = r_tile.rearrange("p (gd e) -> p gd e", e=DIM)
                    off = 0
                    w = DIM // 2
                    for lv in range(5):
                        if lv == 4:
                            out_ap = rh.rearrange("p (gd e) -> p gd e", e=1)
                        else:
                            out_ap = tscr[:, off : off + 128 * w].rearrange(
                                "p (gd e) -> p gd e", e=w
                            )
                        nc.vector.tensor_add(
                            out_ap, lv_in[:, :, :w], lv_in[:, :, w : 2 * w]
                        )
                        lv_in = out_ap
                        off += 128 * w
                        w //= 2

                    # features F [(m,bp), (g, f, d)]
                    f_blk = fpool.tile([128, NG * 4 * DIM], BF16, tag="F")
                    f4 = f_blk.rearrange("p (g f d) -> p g f d", g=NG, f=4)
                    rh3 = rh.rearrange("p (g d) -> p g d", g=NG)
                    vr3 = v_rep[:, blk * NG * DIM : (blk + 1) * NG * DIM].rearrange(
                        "p (g d) -> p g d", g=NG
                    )
                    mr3 = M_rep[:, blk * NG * DIM : (blk + 1) * NG * DIM].rearrange(
                        "p (g d) -> p g d", g=NG
                    )
                    nc.vector.tensor_mul(f4[:, :, 0, :], rh3, vr3)
                    nc.vector.tensor_mul(f4[:, :, 1, :], rh3, mr3)
                    nc.vector.tensor_sub(f4[:, :, 2, :], rh3, vr3)
                    nc.vector.tensor_sub(f4[:, :, 3, :], rh3, mr3)
                    nc.scalar.activation(f4[:, :, 2, :], f4[:, :, 2, :], AF.Abs)
                    nc.scalar.activation(f4[:, :, 3, :], f4[:, :, 3, :], AF.Abs)

                    # transpose to z^T [(f,d), (g,bp,m)]
                    zt_ps = pp_z.tile([D4, ROWS], BF16, tag="zt")
                    for g in range(NG):
                        nc.tensor.transpose(
                            zt_ps[:, g * 128 : (g + 1) * 128],
                            f_blk[:, g * 128 : (g + 1) * 128],
                            ident_bf,
                        )
                    zt_sb = zpool.tile([D4, ROWS], BF16, tag="zt_sb")
                    nc.scalar.copy(out=zt_sb, in_=zt_ps)

                    ps1 = pp_1.tile([DIM, ROWS], F32, tag="ps1")
                    nc.tensor.matmul(ps1, lhsT=W1T, rhs=zt_sb, start=True, stop=True)
                    a1 = apool.tile([DIM, ROWS], BF16, tag="a1")
                    nc.scalar.activation(a1, ps1, AF.Tanh, bias=b1T)
                    ps2 = pp_2.tile([1, ROWS], F32, tag="ps2")
                    nc.tensor.matmul(ps2, lhsT=W2T, rhs=a1, start=True, stop=True)
                    z_sb = zpool.tile([1, ROWS], F32, tag="z_sb")
                    nc.scalar.copy(out=z_sb, in_=ps2)
                    # z_sb free order is (g, m, bp); store as (bp, g, m).
                    # src stays 1-partition (dim0 count 1): split by bp.
                    for bp in range(2):
                        nc.scalar.dma_start(
                            out=z_scr[blk, bp].unsqueeze(0),
                            in_=z_sb.rearrange("o (g m bp) -> o g m bp", g=NG, bp=2)[
                                :, :, :, bp
                            ],
                        )

                # gather Z rows from DRAM: flat (blk,bp,g) == natural b
                nc.scalar.dma_start(
                    out=Z_row,
                    in_=z_scr.rearrange("a b c m -> (a b c) m"),
                )

                # softmax over m, batched over all 128 b
                nmx = small.tile([BC, 1], F32, tag="nmx")
                nc.vector.tensor_reduce(
                    out=nmx, in_=Z_row, axis=AX.X, op=ALU.max, negate=True
                )
                e_row = small.tile([BC, N_MEM], F32, tag="e_row")
                nc.scalar.activation(e_row, Z_row, AF.Exp, bias=nmx)
                ssum = small.tile([BC, 1], F32, tag="ssum")
                nc.vector.tensor_reduce(out=ssum, in_=e_row, axis=AX.X, op=ALU.add)
                rsum = small.tile([BC, 1], F32, tag="rsum")
                nc.vector.reciprocal(rsum, ssum)
                g_row = small.tile([BC, N_MEM], F32, tag="g_row")
                nc.vector.tensor_scalar_mul(g_row, e_row, rsum)
                g_bf = small.tile([BC, N_MEM], BF16, tag="g_bf")
                nc.scalar.copy(out=g_bf, in_=g_row)

                # o[b,d] = sum_m t[b,d,m] * g[b,m]  (in-place mul into t_hop)
                t3 = t_hop.rearrange("b (d m) -> b d m", d=DIM)
                g3 = g_bf.unsqueeze(1).broadcast_to((BC, DIM, N_MEM))
                nc.vector.tensor_mul(t3, t3, g3)
                o_row = small.tile([BC, DIM], F32, tag="o_row")
                nc.vector.tensor_reduce(
                    out=o_row, in_=t3, axis=AX.X, op=ALU.add
                )

                # GRU (transposed layout [*, b], f32)
                ot_ps = pp_g.tile([DIM, BC], F32, tag="gpsum")
                nc.tensor.transpose(ot_ps, o_row, ident)
                oT = small.tile([DIM, BC], F32, tag="oT")
                nc.scalar.copy(out=oT, in_=ot_ps)

                # per-gate matmuls so every gate tile sits at base partition 0
                def gate_pair(g):
                    gi = pp_g.tile([DIM, BC], F32, tag="gpsum")
                    nc.tensor.matmul(
                        gi,
                        lhsT=WihT[hop][:, g * DIM : (g + 1) * DIM],
                        rhs=oT,
                        start=True,
                        stop=True,
                    )
                    gh = pp_g.tile([DIM, BC], F32, tag="gpsum")
                    nc.tensor.matmul(
                        gh,
                        lhsT=WhhT[hop][:, g * DIM : (g + 1) * DIM],
                        rhs=MT,
                        start=True,
                        stop=True,
                    )
                    return gi, gh

                # r,z gates: sigmoid(gi + gh + b_ih + b_hh)
                rz_t = []
                for g in range(2):
                    gi, gh = gate_pair(g)
                    gb = small.tile([DIM, BC], F32, tag=f"g{g}b")
                    nc.scalar.activation(gb, gi, AF.Identity, bias=bsum_rz[hop][g])
                    nc.vector.tensor_add(gb, gb, gh)
                    gt = small.tile([DIM, BC], F32, tag=f"gate{g}")
                    nc.scalar.activation(gt, gb, AF.Sigmoid)
                    rz_t.append(gt)
                r_t, z_t = rz_t

                # n = tanh(gi_n + b_ih_n + r * (gh_n + b_hh_n))
                gi_n, gh_n = gate_pair(2)
                ghn = small.tile([DIM, BC], F32, tag="ghn")
                nc.scalar.activation(ghn, gh_n, AF.Identity, bias=bhhn_t[hop])
                gin = small.tile([DIM, BC], F32, tag="gin")
                nc.scalar.activation(gin, gi_n, AF.Identity, bias=bihn_t[hop])
                n1 = small.tile([DIM, BC], F32, tag="n1")
                nc.vector.tensor_mul(n1, r_t, ghn)
                nc.vector.tensor_add(n1, n1, gin)
                n_t = small.tile([DIM, BC], F32, tag="n_t")
                nc.scalar.activation(n_t, n1, AF.Tanh)

                # M' = n + z * (M - n)
                MT_new = mstate.tile([DIM, BC], F32, tag="MT")
                nc.vector.tensor_sub(MT_new, MT, n_t)
                nc.vector.tensor_mul(MT_new, MT_new, z_t)
                nc.vector.tensor_add(MT_new, MT_new, n_t)
                MT = MT_new

                # M_row for output / M_rep rebuild
                mrow_ps = pp_g.tile([BC, DIM], F32, tag="gpsum")
                nc.tensor.transpose(mrow_ps, MT, ident[:DIM, :DIM])
                M_row = mstate.tile([BC, DIM], F32, tag="M_row")
                nc.scalar.copy(out=M_row, in_=mrow_ps)

                if hop < N_HOP - 1:
                    # rebuild M_rep via DRAM bounce (bf16)
                    M_row_bf = mstate.tile([BC, DIM], BF16, tag="M_row_bf")
                    nc.scalar.copy(out=M_row_bf, in_=mrow_ps)
                    nc.scalar.dma_start(out=m_scr[:, :], in_=M_row_bf)
                    nc.sync.dma_start(
                        out=m_scr2[:, :, :, :],
                        in_=m_scr.rearrange(
                            "(blk bp g) d -> bp blk g d", bp=2, g=NG
                        ),
                    )
                    M_rep_new = mstate.tile(
                        [128, NBLK * NG * DIM], BF16, tag="M_rep", bufs=1
                    )
                    nc.sync.dma_start(
                        out=m_scr3.rearrange("(m bp) f -> m bp f", bp=2),
                        in_=m_scr2.rearrange(
                            "bp blk g d -> bp (blk g d)"
                        ).partition_broadcast(64),
                    )
                    prev_bcast_dma = nc.sync.dma_start(
                        out=M_rep_new, in_=m_scr3[:, :]
                    )
                    M_rep = M_rep_new
                else:
                    nc.sync.dma_start(out=out_d[:, :], in_=M_row)

    nc.compile()
    return nc


_NC_CACHE = None


def _get_nc():
    global _NC_CACHE
    if _NC_CACHE is None:
        _NC_CACHE = build_nc()
    return _NC_CACHE


BF = ml_dtypes.bfloat16


def permute_local(x):
    """[BC, N_HOP, m, ...] -> [N_HOP, NBLK, m, 2, NG, ...] with b = blk*8+bp*4+g."""
    tail = x.shape[2:]
    y = x.reshape(NBLK, 2, NG, N_HOP, *tail)
    order = (3, 0, 4, 1, 2) + tuple(range(5, y.ndim))
    return np.ascontiguousarray(y.transpose(order))


def permute_h(x):
    """hs [BC, N_HOP, m, e] -> [N_HOP, m, 2, NBLK, NG, e]."""
    y = x.reshape(NBLK, 2, NG, N_HOP, N_MEM, DIM)
    return np.ascontiguousarray(y.transpose(3, 4, 1, 0, 2, 5))


def make_in_maps(hs, Rs, ts, vs, W1, b1, W2, W_ih, W_hh, b_ih, b_hh):
    W1T = np.ascontiguousarray(W1.T.astype(BF))
    W2T = np.ascontiguousarray(W2.T.astype(BF))
    WihT = np.ascontiguousarray(W_ih.transpose(0, 2, 1))
    WhhT = np.ascontiguousarray(W_hh.transpose(0, 2, 1))
    in_maps = []
    for c in range(N_CORES):
        sl = slice(c * BC, (c + 1) * BC)
        vsc = vs[sl]
        # vs_rep[(m,bp), (blk,g,d)] = vs[blk*8+bp*4+g, d]
        v4 = vsc.reshape(NBLK, 2, NG, DIM).transpose(1, 0, 2, 3)  # [bp,blk,g,d]
        vs_rep = np.broadcast_to(
            v4.reshape(1, 2, NBLK * NG * DIM), (N_MEM, 2, NBLK * NG * DIM)
        ).reshape(128, NBLK * NG * DIM)
        in_maps.append(
            {
                "Rs": permute_local(Rs[sl]).astype(BF),
                "hs": permute_h(hs[sl]).astype(BF),
                "ts": np.ascontiguousarray(
                    ts[sl].transpose(1, 0, 3, 2)
                ).astype(BF),
                "vsT": np.ascontiguousarray(vsc.T),
                "vs_rep": np.ascontiguousarray(vs_rep).astype(BF),
                "W1T": W1T,
                "b1": np.ascontiguousarray(b1),
                "W2T": W2T,
                "WihT": WihT,
                "WhhT": WhhT,
                "b_ih": np.ascontiguousarray(b_ih),
                "b_hh": np.ascontiguousarray(b_hh),
            }
        )
    return in_maps


def kernel(hs, Rs, ts, vs, W1, b1, W2, b2, W_ih, W_hh, b_ih, b_hh):
    from concourse.bass_utils import run_bass_kernel_spmd

    nc = _get_nc()
    in_maps = make_in_maps(hs, Rs, ts, vs, W1, b1, W2, W_ih, W_hh, b_ih, b_hh)
    res = run_bass_kernel_spmd(nc, in_maps, list(range(N_CORES)))
    return np.concatenate([r["out"] for r in res.results], axis=0)
